# revision 44
# baseline (speedup 1.0000x reference)
"""Trainium2 Bass kernel for ModalityAttention (B=4, S=1024, D=2048, H=16, HD=128, RD=64).

Sharding: 8 cores = 4 batches x 2 head-groups (8 heads each).
Each core computes, for its (batch b, head-group g):
  layernorm(x[b]) -> modulation (scale/bias precomputed on host from mod@mod_w)
  -> qkv projection for its 8 heads -> rmsnorm + rope -> attention
  -> partial out-projection (transposed layout) with gate folded in.
Host gathers: out[b] = (partial_g0 + partial_g1).T + x[b]
(residual added on host; vb = out_b*gate folded into the g0 partial on device).

Precision strategy (correctness gate is rel_err < 2e-2; measured ~3e-3):
  - projections in fp16 (1 PE cycle/row, fp32 PSUM accumulate). fp8 projections
    were tried and fail the gate: peaked attention rows pass per-token q/k/v
    dot-product noise straight through (measured 1.8e-2 from the v projection
    alone).
  - attention weights exp() stored fp8e4m3 with a constant -2 shift (cancels in
    softmax normalization); v stored fp8 after the fp16 matmul; attn@V and the
    softmax denominator both run in fp8 DoubleRow (0.5 cycles/row, K=256/instr)
    over m-tile pairs. The denominator uses an all-ones fp8 stationary of full
    128-column width, which makes the PE replicate the per-query sums across
    all partitions -- no partition broadcast needed.
  - rope pair layout permuted on host (even/odd hd columns of wq/wk separated)
    so rope is contiguous-vector math; scores are permutation-invariant
"""
import os, sys

for _p in ("/opt/trn_rl_repo", "/root/.axon_site/_ro/trn_rl_repo", "/root/.axon_site"):
    if os.path.isdir(_p) and _p not in sys.path:
        sys.path.insert(0, _p)

import numpy as np
import ml_dtypes
import concourse.bass as bass
import concourse.bacc as bacc
import concourse.mybir as mybir
import concourse.tile as tile
from concourse import bass_isa
from concourse.masks import make_identity
from concourse.bass_utils import run_bass_kernel_spmd

F32 = mybir.dt.float32
F16 = mybir.dt.float16
F8 = mybir.dt.float8e4
DR = mybir.MatmulPerfMode.DoubleRow
AF = mybir.ActivationFunctionType
OP = mybir.AluOpType
S, D, HG, HD, RD = 1024, 2048, 8, 128, 64
NT = S // 128        # 8 s-tiles
KT = D // 128        # 16 d-tiles
KP = KT // 2         # 8 d-tile pairs (DoubleRow)
MP = NT // 2         # 4 s-tile pairs (DoubleRow attn@V)
GCOLS = HG * HD      # 1024 columns per group per projection
EPS = 1e-6
WSCALE = 128.0       # host premultiplier on fp8 qkv weights
ESHIFT = -2.0        # exp(score - 2): keeps attn weights under fp8e4m3 max
N_CORES = 8
NPF8 = ml_dtypes.float8_e4m3


def _bcast_from_dram(ap, parts, reps=None):
    """DRAM AP -> partition-broadcast (and optional middle-dim repeat) source AP."""
    newap = [[0, parts]]
    if reps is not None:
        newap.append([0, reps])
    newap += list(ap.ap)
    return bass.AP(tensor=ap.tensor, offset=ap.offset, ap=newap)


def build_nc(has_qkv_bias: bool, has_norm_w: bool):
    nc = bacc.Bacc("TRN2", target_bir_lowering=False, debug=False,
                   enable_asserts=True, num_devices=N_CORES)

    x = nc.dram_tensor("x", [S, D], F16, kind="ExternalInput").ap()
    cos = nc.dram_tensor("cos", [S, RD // 2], F16, kind="ExternalInput").ap()
    sin = nc.dram_tensor("sin", [S, RD // 2], F16, kind="ExternalInput").ap()
    # qkv weights: host-prepacked [k-tile, partition(d%128), col], fp16
    wq = nc.dram_tensor("wq", [KT, 128, GCOLS], F16, kind="ExternalInput").ap()
    wk = nc.dram_tensor("wk", [KT, 128, GCOLS], F16, kind="ExternalInput").ap()
    wv = nc.dram_tensor("wv", [KT, 128, GCOLS], F16, kind="ExternalInput").ap()
    # out-proj weights: host-prepacked [m(d-tile), partition(hd), kb*128+col]
    wo = nc.dram_tensor("wo", [KT, 128, GCOLS], F16, kind="ExternalInput").ap()
    # modulation vectors, pre-reshaped on host to [128, KT] (column k = d-tile k)
    scale1p = nc.dram_tensor("scale1p", [128, KT], F32, kind="ExternalInput").ap()
    biasm = nc.dram_tensor("biasm", [128, KT], F32, kind="ExternalInput").ap()
    gate = nc.dram_tensor("gate", [128, KT], F32, kind="ExternalInput").ap()
    vb = nc.dram_tensor("vb", [128, KT], F32, kind="ExternalInput").ap()
    if has_qkv_bias:
        bq = nc.dram_tensor("bq", [GCOLS], F16, kind="ExternalInput").ap()
        bk = nc.dram_tensor("bk", [GCOLS], F16, kind="ExternalInput").ap()
        bv = nc.dram_tensor("bv", [GCOLS], F16, kind="ExternalInput").ap()
    if has_norm_w:
        wqn = nc.dram_tensor("wqn", [HD], F16, kind="ExternalInput").ap()
        wkn = nc.dram_tensor("wkn", [HD], F16, kind="ExternalInput").ap()
    out_t = nc.dram_tensor("out_t", [D, S], F16, kind="ExternalOutput").ap()
    out_r = out_t.rearrange("(m p) s -> p m s", p=128)

    with tile.TileContext(nc) as tc, \
            nc.allow_low_precision(reason="fp16/fp8 attn; tol is 2e-2"):
        # ======== LEFT stack bottom: small persistent constants ====================
        misc_cm = tc.tile_pool(name="misc", bufs=1, side="left")
        misc = misc_cm.__enter__()
        ident = misc.tile([128, 128], F16)
        make_identity(nc, ident)
        ones8 = misc.tile([128, 2, 128], F8)
        nc.vector.memset(ones8, 1.0)
        eps_t = misc.tile([128, 1], F32)
        nc.vector.memset(eps_t, EPS)
        eshift_t = misc.tile([128, 1], F32)
        nc.vector.memset(eshift_t, ESHIFT)
        eps128_t = misc.tile([128, 1], F32)
        nc.vector.memset(eps128_t, HD * EPS)
        gate_sb = misc.tile([128, KT], F32)
        vb_sb = misc.tile([128, KT], F32)
        rrk_all = misc.tile([128, NT, HG], F32)   # scaled k-rms reciprocals
        if has_norm_w:
            wqn_b = misc.tile([128, HG, HD], F16)
            wkn_b = misc.tile([128, HG, HD], F16)
        cs_tiles = []
        for m in range(NT):
            ct = misc.tile([128, RD // 2], F16, tag=f"cos_{m}", name=f"cos_{m}")
            st = misc.tile([128, RD // 2], F16, tag=f"sin_{m}", name=f"sin_{m}")
            cs_tiles.append((ct, st))

        # ======== RIGHT stack: big natural-layout tensors (B..E lifetimes) =========
        v_cm = tc.tile_pool(name="vpool", bufs=1, side="right")
        v_p = v_cm.__enter__()
        vnat = v_p.tile([128, NT, GCOLS], F8)
        if has_qkv_bias:
            vnat16 = v_p.tile([128, NT, GCOLS], F16)
        natqk_cm = tc.tile_pool(name="natqk", bufs=1, side="right")
        natqk = natqk_cm.__enter__()
        qnat = natqk.tile([128, NT, GCOLS], F16)
        knat = natqk.tile([128, NT, GCOLS], F16)

        # ======== phase A: layernorm + modulation + transpose -> xnT (fp8) =========
        xnT_cm = tc.tile_pool(name="xnT", bufs=1, side="left")
        xnT_p = xnT_cm.__enter__()
        xnT = xnT_p.tile([128, KT, S], F16)  # [d_in_tile, d_tile, s]

        avec_cm = tc.tile_pool(name="phA_vec", bufs=1, side="left")
        avec = avec_cm.__enter__()
        s1pc = avec.tile([128, KT], F32)
        bmc = avec.tile([128, KT], F32)
        if has_qkv_bias:
            bq_b = avec.tile([128, GCOLS], F16)
            nc.sync.dma_start(out=bq_b, in_=_bcast_from_dram(bq, 128))
            bk_b = avec.tile([128, GCOLS], F16)
            nc.sync.dma_start(out=bk_b, in_=_bcast_from_dram(bk, 128))
            bv_b = avec.tile([128, GCOLS], F16)
            nc.sync.dma_start(out=bv_b, in_=_bcast_from_dram(bv, 128))

        xall_cm = tc.tile_pool(name="phA_x", bufs=1, side="left")
        xall = xall_cm.__enter__()
        xts = []
        for i in range(NT):
            xt = xall.tile([128, D], F16, tag=f"xt{i}", name=f"xt{i}")
            nc.sync.dma_start(out=xt, in_=x[i * 128:(i + 1) * 128, :])
            xts.append(xt)
        nc.sync.dma_start(out=s1pc, in_=scale1p)
        nc.sync.dma_start(out=bmc, in_=biasm)

        a_small_cm = tc.tile_pool(name="phA_small", bufs=4, side="left")
        a_small = a_small_cm.__enter__()
        for i in range(NT):
            xt = xts[i]
            stats = a_small.tile([128, 4, 6], F32, tag="stats")
            xv = xt.rearrange("p (c f) -> p c f", c=4)
            for c in range(4):
                nc.vector.bn_stats(out=stats[:, c, :], in_=xv[:, c, :])
            mv = a_small.tile([128, 2], F32, tag="mv")
            nc.vector.bn_aggr(out=mv, in_=stats)
            rstd = a_small.tile([128, 1], F32, tag="rstd")
            nc.scalar.activation(out=rstd, in_=mv[:, 1:2], func=AF.Sqrt,
                                 bias=eps_t, scale=1.0)
            nc.vector.reciprocal(out=rstd, in_=rstd)
            # x_norm = (x - mean) * rstd, in place on DVE (fp16 2x mode)
            nc.vector.tensor_scalar(out=xt, in0=xt, scalar1=mv[:, 0:1],
                                    scalar2=rstd, op0=OP.subtract, op1=OP.mult)

        # transposes, i-major over two 8-bank halves: each i-pass runs as soon
        # as tile i's layernorm lands, so the in-order PE tracks the DVE
        # instead of blocking on the last tile. 8 s-blocks of one d-tile pack
        # into one PSUM bank; evacs alternate ACT/DVE.
        pst_cm = tc.tile_pool(name="ps_tr", bufs=1, space="PSUM")
        pst = pst_cm.__enter__()
        for half in range(2):
            pts = [pst.tile([128, NT, 128], F16, tag=f"pt{kk}", name=f"pt{kk}")
                   for kk in range(8)]
            for i in range(NT):
                for kk in range(8):
                    k = half * 8 + kk
                    nc.tensor.matmul(pts[kk][:, i, :],
                                     xts[i][:, k * 128:(k + 1) * 128],
                                     ident, is_transpose=True,
                                     start=(i == 0), stop=(i == NT - 1),
                                     skip_group_check=True)
            # modulation fused into the evac: xnT = pt * (1+scale[d]) + bias[d]
            for kk in range(8):
                k = half * 8 + kk
                ptf = pts[kk].rearrange("p a b -> p (a b)")
                if kk % 2 == 0:
                    nc.scalar.activation(out=xnT[:, k, :], in_=ptf,
                                         func=AF.Identity,
                                         bias=bmc[:, k:k + 1],
                                         scale=s1pc[:, k:k + 1])
                else:
                    nc.vector.tensor_scalar(out=xnT[:, k, :], in0=ptf,
                                            scalar1=s1pc[:, k:k + 1],
                                            scalar2=bmc[:, k:k + 1],
                                            op0=OP.mult, op1=OP.add)
        pst_cm.__exit__(None, None, None)
        a_small_cm.__exit__(None, None, None)
        xall_cm.__exit__(None, None, None)

        # phase C pools opened BEFORE phase B emission so the rms/rope work can
        # overlap the v-projection matmuls (no pool-boundary serialization).
        c_cm = tc.tile_pool(name="phC", bufs=2, side="left")
        c_p = c_cm.__enter__()
        c_small_cm = tc.tile_pool(name="phC_small", bufs=2, side="left")
        c_small = c_small_cm.__enter__()

        # ======== phase B: qkv projections (fp16) ==================================
        wv_cm = tc.tile_pool(name="wv_res", bufs=1, side="right")
        wv_p = wv_cm.__enter__()
        w_cm = tc.tile_pool(name="wstream", bufs=1, side="right")
        w_p = w_cm.__enter__()
        psb_cm = tc.tile_pool(name="ps_qkv", bufs=1, space="PSUM")
        psb = psb_cm.__enter__()

        def emit_proj(wdram, evacs, morder=None, after_n1_evac=None, wtag="wt"):
            """m-major: each m's K-chain completes early so its consumer
            (PSUM evac + per-m epilogue) pipelines with the next chain."""
            wts = []
            for k in range(KT):
                wt = w_p.tile([128, GCOLS], F16, tag=f"{wtag}{k}", name=f"{wtag}{k}")
                nc.sync.dma_start(out=wt, in_=wdram[k])
                wts.append(wt)
            for n in range(2):
                for m in (morder or range(NT)):
                    ps = psb.tile([128, 512], F32, tag=f"ps{m}", name=f"ps{m}")
                    for k in range(KT):
                        nc.tensor.matmul(
                            ps, xnT[:, k, m * 128:(m + 1) * 128],
                            wts[k][:, n * 512:(n + 1) * 512],
                            start=(k == 0), stop=(k == KT - 1))
                    evacs(n, m, ps)
                    if n == 1 and after_n1_evac is not None:
                        after_n1_evac(m)

        # ======== phase C: rmsnorm + rope on q, k (natural, in place) ==============
        # C(m) is emitted immediately after k-tile m's last PSUM evacuation, and
        # its work is split DVE/GPSIMD, so it overlaps the B matmuls instead of
        # serializing behind them on one engine.
        def emit_c(m):
            qm = qnat[:, m, :]
            km = knat[:, m, :]
            (ct, st) = cs_tiles[m]
            cb = ct.unsqueeze(1).broadcast_to([128, HG, RD // 2])
            sb_ = st.unsqueeze(1).broadcast_to([128, HG, RD // 2])
            if has_qkv_bias:
                nc.vector.tensor_add(out=qm, in0=qm, in1=bq_b)
                nc.vector.tensor_add(out=km, in0=km, in1=bk_b)

            # rms stats (on raw q/k, before norm-w and rope)
            sq = c_p.tile([128, GCOLS], F16, tag="sqk")
            nc.vector.tensor_mul(out=sq, in0=qm, in1=qm)
            ssq = c_small.tile([128, HG], F16, tag="ssq")
            nc.vector.reduce_sum(out=ssq, in_=sq.rearrange("p (h d) -> p h d", h=HG),
                                 axis=mybir.AxisListType.X)
            rrq = c_small.tile([128, HG], F32, tag="rrq")
            nc.scalar.activation(out=rrq, in_=ssq, func=AF.Sqrt,
                                 bias=eps_t, scale=1.0 / HD)
            nc.vector.reciprocal(out=rrq, in_=rrq)
            rrq16 = c_small.tile([128, HG], F16, tag="rrq16")
            nc.vector.tensor_copy(out=rrq16, in_=rrq)

            sk_ = c_p.tile([128, GCOLS], F16, tag="sqk")
            nc.vector.tensor_mul(out=sk_, in0=km, in1=km)
            ssk = c_small.tile([128, HG], F16, tag="ssk")
            nc.vector.reduce_sum(out=ssk, in_=sk_.rearrange("p (h d) -> p h d", h=HG),
                                 axis=mybir.AxisListType.X)
            nc.scalar.activation(out=rrk_all[:, m, :], in_=ssk, func=AF.Sqrt,
                                 bias=eps128_t, scale=1.0)
            nc.vector.reciprocal(out=rrk_all[:, m, :], in_=rrk_all[:, m, :])

            if has_norm_w:
                nc.vector.tensor_mul(out=qm.rearrange("p (h d) -> p h d", h=HG),
                                     in0=qm.rearrange("p (h d) -> p h d", h=HG),
                                     in1=wqn_b)
                nc.vector.tensor_mul(out=km.rearrange("p (h d) -> p h d", h=HG),
                                     in0=km.rearrange("p (h d) -> p h d", h=HG),
                                     in1=wkn_b)

            # rope with host-permuted pair layout: x0 = cols [0:32), x1 = [32:64)
            # of each head (evens/odds separated on host; scores invariant).
            # q rope on DVE, k rope on GPSIMD (runs in parallel).
            for mm, eng in ((qm, nc.vector), (km, nc.gpsimd)):
                mv_ = mm.rearrange("p (h d) -> p h d", h=HG)
                x0 = mv_[:, :, 0:RD // 2]
                x1 = mv_[:, :, RD // 2:RD]
                tg = "q" if eng is nc.vector else "k"
                t0 = c_small.tile([128, HG, RD // 2], F16, tag=f"t0{tg}")
                t1 = c_small.tile([128, HG, RD // 2], F16, tag=f"t1{tg}")
                t2 = c_small.tile([128, HG, RD // 2], F16, tag=f"t2{tg}")
                t3 = c_small.tile([128, HG, RD // 2], F16, tag=f"t3{tg}")
                eng.tensor_mul(out=t0, in0=x0, in1=cb)
                eng.tensor_mul(out=t1, in0=x1, in1=sb_)
                eng.tensor_mul(out=t2, in0=x0, in1=sb_)
                eng.tensor_mul(out=t3, in0=x1, in1=cb)
                eng.tensor_sub(out=x0, in0=t0, in1=t1)
                eng.tensor_add(out=x1, in0=t2, in1=t3)

            # apply q rms reciprocal (k's is folded into the exp scale later)
            rrq_b = rrq16.unsqueeze(2).broadcast_to([128, HG, HD])
            nc.gpsimd.tensor_mul(out=qm.rearrange("p (h d) -> p h d", h=HG),
                                 in0=qm.rearrange("p (h d) -> p h d", h=HG),
                                 in1=rrq_b)

        # rotate the first group's m-order: ps[0]/ps[1] reuse the PSUM banks of
        # the last two transpose tiles, whose evacs land latest
        emit_proj(wq, lambda n, m, ps: nc.scalar.copy(
            out=qnat[:, m, n * 512:(n + 1) * 512], in_=ps), wtag="wq")

        # deferred misc loads (consumed in phases C/E/F) -- emitted after the
        # q weight stream so their HWDGE overhead doesn't delay phase B's start
        nc.sync.dma_start(out=gate_sb, in_=gate)
        nc.sync.dma_start(out=vb_sb, in_=vb)
        if has_norm_w:
            nc.sync.dma_start(out=wqn_b, in_=_bcast_from_dram(wqn, 128, reps=HG))
            nc.sync.dma_start(out=wkn_b, in_=_bcast_from_dram(wkn, 128, reps=HG))
        for m in range(NT):
            ct, st = cs_tiles[m]
            nc.sync.dma_start(out=ct, in_=cos[m * 128:(m + 1) * 128, :])
            nc.sync.dma_start(out=st, in_=sin[m * 128:(m + 1) * 128, :])

        emit_proj(wk, lambda n, m, ps: nc.scalar.copy(
            out=knat[:, m, n * 512:(n + 1) * 512], in_=ps),
            after_n1_evac=emit_c, wtag="wk")

        if has_qkv_bias:
            emit_proj(wv, lambda n, m, ps: nc.scalar.copy(
                out=vnat16[:, m, n * 512:(n + 1) * 512], in_=ps), wtag="wv")
            for m in range(NT):
                nc.vector.tensor_add(out=vnat16[:, m, :], in0=vnat16[:, m, :],
                                     in1=bv_b)
                nc.vector.tensor_copy(out=vnat[:, m, :], in_=vnat16[:, m, :])
            wvts = None
        else:
            # v weights stay resident; the v projection is emitted later as
            # 2-PSUM-bank m-serial chains (n=0 before phase D, n=1 woven into
            # phase E's head loop to fill PE stalls during the exp waits).
            wvts = []
            for k in range(KT):
                wt = wv_p.tile([128, GCOLS], F16, tag=f"wv{k}", name=f"wv{k}")
                nc.sync.dma_start(out=wt, in_=wv[k])
                wvts.append(wt)

        psb_cm.__exit__(None, None, None)
        w_cm.__exit__(None, None, None)

        psbv_cm = tc.tile_pool(name="ps_v", bufs=2, space="PSUM")
        psbv = psbv_cm.__enter__()

        def bv_thunks(n):
            """Yield emission thunks for the v projection of one n-half:
            per m, a 16-matmul K-chain into a single PSUM bank + DVE evac."""
            for m in range(NT):
                psv = psbv.tile([128, 512], F32, tag="psv")
                for k in range(KT):
                    yield (lambda psv=psv, k=k, m=m, n=n: nc.tensor.matmul(
                        psv, xnT[:, k, m * 128:(m + 1) * 128],
                        wvts[k][:, n * 512:(n + 1) * 512],
                        start=(k == 0), stop=(k == KT - 1)))
                # n=0 evacs on ACT (idle pre-E); n=1 evacs on DVE (idle in D)
                if n == 0:
                    yield (lambda psv=psv, m=m, n=n: nc.scalar.copy(
                        out=vnat[:, m, n * 512:(n + 1) * 512], in_=psv))
                else:
                    yield (lambda psv=psv, m=m, n=n: nc.vector.tensor_copy(
                        out=vnat[:, m, n * 512:(n + 1) * 512], in_=psv))

        if not has_qkv_bias:
            for t in bv_thunks(0):
                t()
            bv_rest = bv_thunks(1)
        else:
            bv_rest = iter(())

        def drain(nthunks):
            for _ in range(nthunks):
                t = next(bv_rest, None)
                if t is None:
                    return
                t()

        # ======== phase D: transpose q, k -> [hd, s] per head ======================
        oT_cm = tc.tile_pool(name="oT", bufs=1, side="left")
        oT_p = oT_cm.__enter__()
        oT = oT_p.tile([128, HG, S], F16)

        qkT_cm = tc.tile_pool(name="qkT", bufs=1, side="left")
        qkT_p = qkT_cm.__enter__()
        qT = qkT_p.tile([128, HG, S], F16)
        kT = qkT_p.tile([128, HG, S], F16)

        # the v projection's n=1 half is woven between the transpose groups:
        # the PE is otherwise evac-paced here, and psbv (2 banks) + pst2 (4)
        # fit alongside each other before the attention pools open
        pst2_cm = tc.tile_pool(name="ps_tr2", bufs=4, space="PSUM")
        pst2 = pst2_cm.__enter__()
        for h in range(HG):
            for (nat, dst, evac) in (
                    (qnat, qT, lambda o, i: nc.scalar.copy(out=o, in_=i)),
                    (knat, kT, lambda o, i: nc.vector.tensor_copy(out=o, in_=i))):
                pt2 = pst2.tile([128, NT, 128], F16, tag="pt2")
                for m in range(NT):
                    nc.tensor.matmul(pt2[:, m, :], nat[:, m, h * 128:(h + 1) * 128],
                                     ident, is_transpose=True,
                                     start=(m == 0), stop=(m == NT - 1),
                                     skip_group_check=True)
                drain(9)
                evac(dst[:, h, :], pt2.rearrange("p a b -> p (a b)"))
        drain(1 << 30)
        pst2_cm.__exit__(None, None, None)
        psbv_cm.__exit__(None, None, None)
        wv_cm.__exit__(None, None, None)
        natqk_cm.__exit__(None, None, None)

        # ======== phase E: attention per head ======================================
        # F's weight pool opens before E so all wo tiles stream during E
        # (one slot per tile: a shared slot would head-of-line-block the DMA
        # queue behind F's progress)
        f_cm = tc.tile_pool(name="phF", bufs=2, side="left")
        f_p = f_cm.__enter__()
        wo_ts = []
        for mp in range(KT // 2):
            wo_t = f_p.tile([128, 2, GCOLS], F16, tag=f"wo_t{mp}",
                            name=f"wo_t{mp}")
            nc.sync.dma_start(
                out=wo_t, in_=wo.rearrange("m p c -> p m c")[:, 2 * mp:2 * mp + 2, :])
            wo_ts.append(wo_t)

        at_cm = tc.tile_pool(name="attn", bufs=2, side="left")
        at_p = at_cm.__enter__()
        rs_cm = tc.tile_pool(name="rsb", bufs=2, side="left")
        rs_p = rs_cm.__enter__()
        # pool order: o/den first so the score tiles land on banks whose last
        # writers (phase D transposes) retire earliest
        pso_cm = tc.tile_pool(name="ps_o", bufs=1, space="PSUM")
        pso = pso_cm.__enter__()
        psd_cm = tc.tile_pool(name="ps_den", bufs=1, space="PSUM")
        psd = psd_cm.__enter__()
        pssc_cm = tc.tile_pool(name="ps_sc", bufs=2, space="PSUM")
        pssc = pssc_cm.__enter__()

        for h in range(HG):
            o_ps = pso.tile([128, S], F32, tag="o_ps")
            # ones-matmul denominator: every partition gets the same column sums,
            # so no partition broadcast is needed afterwards
            den_ps = psd.tile([128, 2, 512], F32, tag="den_ps")
            at2 = None
            sc_next = pssc.tile([128, S], F32, tag="sc")
            nc.tensor.matmul(sc_next[:, 0:512], kT[:, h, 0:128], qT[:, h, 0:512],
                             start=True, stop=True)
            nc.tensor.matmul(sc_next[:, 512:1024], kT[:, h, 0:128],
                             qT[:, h, 512:1024], start=True, stop=True)
            for m in range(NT):
                sc = sc_next
                if m < NT - 1:
                    sc_next = pssc.tile([128, S], F32, tag="sc")
                    lhs_k = kT[:, h, (m + 1) * 128:(m + 2) * 128]
                    nc.tensor.matmul(sc_next[:, 0:512], lhs_k, qT[:, h, 0:512],
                                     start=True, stop=True)
                    nc.tensor.matmul(sc_next[:, 512:1024], lhs_k,
                                     qT[:, h, 512:1024], start=True, stop=True)
                if m % 2 == 0:
                    at2 = at_p.tile([128, 2, S], F8, tag="at2", name="at2")
                # attn weights: exp(score/rms - 2), fp8 (shift cancels in softmax)
                nc.scalar.activation(out=at2[:, m % 2, :], in_=sc, func=AF.Exp,
                                     bias=eshift_t, scale=rrk_all[:, m, h:h + 1])
                if m % 2 == 1:
                    mp = m // 2
                    first, last = (mp == 0), (mp == MP - 1)
                    v_mh = vnat[:, m - 1:m + 1, h * 128:(h + 1) * 128]
                    for c in range(2):
                        rhs = at2[:, :, c * 512:(c + 1) * 512]
                        nc.tensor.matmul(o_ps[:, c * 512:(c + 1) * 512], v_mh,
                                         rhs, start=first, stop=last, perf_mode=DR)
                        nc.tensor.matmul(den_ps[:, c, :], ones8, rhs,
                                         start=first, stop=last, perf_mode=DR)
            # reciprocal of the (partition-replicated) denominator, normalize
            sums_b = rs_p.tile([128, S], F16, tag="sums_b")
            nc.vector.reciprocal(out=sums_b,
                                 in_=den_ps.rearrange("p a b -> p (a b)"))
            nc.vector.tensor_mul(out=oT[:, h, :], in0=o_ps, in1=sums_b)

        pssc_cm.__exit__(None, None, None)
        psd_cm.__exit__(None, None, None)
        pso_cm.__exit__(None, None, None)
        rs_cm.__exit__(None, None, None)
        at_cm.__exit__(None, None, None)
        v_cm.__exit__(None, None, None)

        # ---- phase F: out projection (transposed out)
        psf_cm = tc.tile_pool(name="ps_out", bufs=2, space="PSUM")
        psf = psf_cm.__enter__()
        for mp in range(KT // 2):
            wo_t = wo_ts[mp]
            ot2 = f_p.tile([128, 2, S], F16, tag="ot2")
            for j in range(2):
                m = 2 * mp + j
                po = psf.tile([128, S], F32, tag="po")
                for kb in range(HG):
                    first, last = (kb == 0), (kb == HG - 1)
                    lhs = wo_t[:, j, kb * 128:(kb + 1) * 128]
                    nc.tensor.matmul(po[:, 0:512], lhs, oT[:, kb, 0:512],
                                     start=first, stop=last)
                    nc.tensor.matmul(po[:, 512:1024], lhs, oT[:, kb, 512:1024],
                                     start=first, stop=last)
                nc.scalar.activation(out=ot2[:, j, :], in_=po, func=AF.Identity,
                                     bias=vb_sb[:, m:m + 1], scale=gate_sb[:, m:m + 1])
            nc.sync.dma_start(out=out_r[:, 2 * mp:2 * mp + 2, :], in_=ot2)
        psf_cm.__exit__(None, None, None)
        f_cm.__exit__(None, None, None)
        qkT_cm.__exit__(None, None, None)
        oT_cm.__exit__(None, None, None)
        c_small_cm.__exit__(None, None, None)
        c_cm.__exit__(None, None, None)
        avec_cm.__exit__(None, None, None)
        xnT_cm.__exit__(None, None, None)
        misc_cm.__exit__(None, None, None)

    nc.compile()
    return nc


_NC_CACHE = {}


def _get_nc(has_qkv_bias, has_norm_w):
    key = (has_qkv_bias, has_norm_w)
    if key not in _NC_CACHE:
        _NC_CACHE[key] = build_nc(*key)
    return _NC_CACHE[key]


# rope pair permutation: within each head's first RD columns, evens then odds.
_ROPE_PERM_HD = np.r_[np.arange(0, RD, 2), np.arange(1, RD, 2), np.arange(RD, HD)]
_ROPE_PERM = np.concatenate([h * HD + _ROPE_PERM_HD for h in range(HG)])


def _pack_qkv_w(w):
    """[D, GCOLS] fp32 -> [KT, 128, GCOLS] fp16."""
    return np.ascontiguousarray(w.reshape(KT, 128, GCOLS).astype(np.float16))


def _pack_wo(w):
    """[GCOLS, D] fp32 -> [KT, 128, HG*128] fp16 (wo_p[m,p,kb*128+j] = w[kb*128+p, m*128+j])."""
    wp = w.reshape(HG, 128, KT, 128).transpose(2, 1, 0, 3).reshape(KT, 128, GCOLS)
    return np.ascontiguousarray(wp.astype(np.float16))


def prep_in_maps(x, mod, cos, sin, qkv_w, qkv_b, mod_w, mod_b, out_w, out_b,
                 norm_q_w, norm_k_w):
    """Host-side sharding. Returns (in_maps, flags, x_np)."""
    x = np.asarray(x, dtype=np.float32)
    m3 = np.asarray(mod, np.float32) @ np.asarray(mod_w, np.float32) \
        + np.asarray(mod_b, np.float32)
    bias, scale, gatef = np.split(m3, 3, axis=-1)          # [B, D] each
    scale1p = (1.0 + scale).astype(np.float32)
    vbf = (np.asarray(out_b, np.float32)[None, :] * gatef).astype(np.float32)

    qkv_b = np.asarray(qkv_b, np.float32)
    has_qkv_bias = bool(np.any(qkv_b != 0.0))
    has_norm_w = not (np.allclose(norm_q_w, 1.0) and np.allclose(norm_k_w, 1.0))

    cosc = np.ascontiguousarray(np.asarray(cos, np.float16))
    sinc = np.ascontiguousarray(np.asarray(sin, np.float16))
    qkv_w = np.asarray(qkv_w, np.float32)
    out_w = np.asarray(out_w, np.float32)
    x16 = x.astype(np.float16)

    in_maps = []
    for c in range(N_CORES):
        b, g = divmod(c, 2)
        lo = g * GCOLS
        im = {
            "x": np.ascontiguousarray(x16[b]),
            "cos": cosc, "sin": sinc,
            "wq": _pack_qkv_w(qkv_w[:, lo:lo + GCOLS][:, _ROPE_PERM]),
            "wk": _pack_qkv_w(qkv_w[:, 2048 + lo:2048 + lo + GCOLS][:, _ROPE_PERM]),
            "wv": _pack_qkv_w(qkv_w[:, 4096 + lo:4096 + lo + GCOLS]),
            "wo": _pack_wo(out_w[lo:lo + GCOLS, :]),
            "scale1p": np.ascontiguousarray(scale1p[b].reshape(KT, 128).T),
            "biasm": np.ascontiguousarray(bias[b].reshape(KT, 128).T),
            "gate": np.ascontiguousarray(gatef[b].reshape(KT, 128).T),
            "vb": np.ascontiguousarray(
                (vbf[b] if g == 0 else np.zeros_like(vbf[b])).reshape(KT, 128).T),
        }
        if has_qkv_bias:
            im["bq"] = np.ascontiguousarray(
                qkv_b[lo:lo + GCOLS][_ROPE_PERM].astype(np.float16))
            im["bk"] = np.ascontiguousarray(
                qkv_b[2048 + lo:2048 + lo + GCOLS][_ROPE_PERM].astype(np.float16))
            im["bv"] = np.ascontiguousarray(
                qkv_b[4096 + lo:4096 + lo + GCOLS].astype(np.float16))
        if has_norm_w:
            im["wqn"] = np.ascontiguousarray(
                np.asarray(norm_q_w, np.float32)[_ROPE_PERM_HD].astype(np.float16))
            im["wkn"] = np.ascontiguousarray(
                np.asarray(norm_k_w, np.float32)[_ROPE_PERM_HD].astype(np.float16))
        in_maps.append(im)
    return in_maps, (has_qkv_bias, has_norm_w), x


def gather(results, x):
    B = x.shape[0]
    outs = []
    for b in range(B):
        p = results[2 * b]["out_t"].astype(np.float32) \
            + results[2 * b + 1]["out_t"].astype(np.float32)   # [D, S]
        outs.append(p.T + x[b])
    return np.stack(outs).astype(np.float32)


def kernel(**inputs) -> np.ndarray:
    in_maps, flags, x = prep_in_maps(**inputs)
    nc = _get_nc(*flags)
    res = run_bass_kernel_spmd(nc, in_maps, core_ids=list(range(N_CORES)))
    return gather(res.results, x)


if __name__ == "__main__":
    import time
    t0 = time.time()
    nc = build_nc(False, False)
    print("build+compile ok in", time.time() - t0, "s")


# revision 45
# speedup vs baseline: 1.0028x; 1.0028x over previous
"""Trainium2 Bass kernel for ModalityAttention (B=4, S=1024, D=2048, H=16, HD=128, RD=64).

Sharding: 8 cores = 4 batches x 2 head-groups (8 heads each).
Each core computes, for its (batch b, head-group g):
  layernorm(x[b]) -> modulation (scale/bias precomputed on host from mod@mod_w)
  -> qkv projection for its 8 heads -> rmsnorm + rope -> attention
  -> partial out-projection (transposed layout) with gate folded in.
Host gathers: out[b] = (partial_g0 + partial_g1).T + x[b]
(residual added on host; vb = out_b*gate folded into the g0 partial on device).

Precision strategy (correctness gate is rel_err < 2e-2; measured ~3e-3):
  - projections in fp16 (1 PE cycle/row, fp32 PSUM accumulate). fp8 projections
    were tried and fail the gate: peaked attention rows pass per-token q/k/v
    dot-product noise straight through (measured 1.8e-2 from the v projection
    alone).
  - attention weights exp() stored fp8e4m3 with a constant -2 shift (cancels in
    softmax normalization); v stored fp8 after the fp16 matmul; attn@V and the
    softmax denominator both run in fp8 DoubleRow (0.5 cycles/row, K=256/instr)
    over m-tile pairs. The denominator uses an all-ones fp8 stationary of full
    128-column width, which makes the PE replicate the per-query sums across
    all partitions -- no partition broadcast needed.
  - rope pair layout permuted on host (even/odd hd columns of wq/wk separated)
    so rope is contiguous-vector math; scores are permutation-invariant
"""
import os, sys

for _p in ("/opt/trn_rl_repo", "/root/.axon_site/_ro/trn_rl_repo", "/root/.axon_site"):
    if os.path.isdir(_p) and _p not in sys.path:
        sys.path.insert(0, _p)

import numpy as np
import ml_dtypes
import concourse.bass as bass
import concourse.bacc as bacc
import concourse.mybir as mybir
import concourse.tile as tile
from concourse import bass_isa
from concourse.masks import make_identity
from concourse.bass_utils import run_bass_kernel_spmd

F32 = mybir.dt.float32
F16 = mybir.dt.float16
F8 = mybir.dt.float8e4
DR = mybir.MatmulPerfMode.DoubleRow
AF = mybir.ActivationFunctionType
OP = mybir.AluOpType
S, D, HG, HD, RD = 1024, 2048, 8, 128, 64
NT = S // 128        # 8 s-tiles
KT = D // 128        # 16 d-tiles
KP = KT // 2         # 8 d-tile pairs (DoubleRow)
MP = NT // 2         # 4 s-tile pairs (DoubleRow attn@V)
GCOLS = HG * HD      # 1024 columns per group per projection
EPS = 1e-6
WSCALE = 128.0       # host premultiplier on fp8 qkv weights
ESHIFT = -2.0        # exp(score - 2): keeps attn weights under fp8e4m3 max
N_CORES = 8
NPF8 = ml_dtypes.float8_e4m3


def _bcast_from_dram(ap, parts, reps=None):
    """DRAM AP -> partition-broadcast (and optional middle-dim repeat) source AP."""
    newap = [[0, parts]]
    if reps is not None:
        newap.append([0, reps])
    newap += list(ap.ap)
    return bass.AP(tensor=ap.tensor, offset=ap.offset, ap=newap)


def build_nc(has_qkv_bias: bool, has_norm_w: bool):
    nc = bacc.Bacc("TRN2", target_bir_lowering=False, debug=False,
                   enable_asserts=True, num_devices=N_CORES)

    x = nc.dram_tensor("x", [S, D], F16, kind="ExternalInput").ap()
    cos = nc.dram_tensor("cos", [S, RD // 2], F16, kind="ExternalInput").ap()
    sin = nc.dram_tensor("sin", [S, RD // 2], F16, kind="ExternalInput").ap()
    # qkv weights: host-prepacked [k-tile, partition(d%128), col], fp16
    wq = nc.dram_tensor("wq", [KT, 128, GCOLS], F16, kind="ExternalInput").ap()
    wk = nc.dram_tensor("wk", [KT, 128, GCOLS], F16, kind="ExternalInput").ap()
    wv = nc.dram_tensor("wv", [KT, 128, GCOLS], F16, kind="ExternalInput").ap()
    # out-proj weights: host-prepacked [m(d-tile), partition(hd), kb*128+col]
    wo = nc.dram_tensor("wo", [KT, 128, GCOLS], F16, kind="ExternalInput").ap()
    # modulation vectors, pre-reshaped on host to [128, KT] (column k = d-tile k)
    scale1p = nc.dram_tensor("scale1p", [128, KT], F32, kind="ExternalInput").ap()
    biasm = nc.dram_tensor("biasm", [128, KT], F32, kind="ExternalInput").ap()
    gate = nc.dram_tensor("gate", [128, KT], F32, kind="ExternalInput").ap()
    vb = nc.dram_tensor("vb", [128, KT], F32, kind="ExternalInput").ap()
    if has_qkv_bias:
        bq = nc.dram_tensor("bq", [GCOLS], F16, kind="ExternalInput").ap()
        bk = nc.dram_tensor("bk", [GCOLS], F16, kind="ExternalInput").ap()
        bv = nc.dram_tensor("bv", [GCOLS], F16, kind="ExternalInput").ap()
    if has_norm_w:
        wqn = nc.dram_tensor("wqn", [HD], F16, kind="ExternalInput").ap()
        wkn = nc.dram_tensor("wkn", [HD], F16, kind="ExternalInput").ap()
    out_t = nc.dram_tensor("out_t", [D, S], F16, kind="ExternalOutput").ap()
    out_r = out_t.rearrange("(m p) s -> p m s", p=128)

    with tile.TileContext(nc) as tc, \
            nc.allow_low_precision(reason="fp16/fp8 attn; tol is 2e-2"):
        # ======== LEFT stack bottom: small persistent constants ====================
        misc_cm = tc.tile_pool(name="misc", bufs=1, side="left")
        misc = misc_cm.__enter__()
        ident = misc.tile([128, 128], F16)
        make_identity(nc, ident)
        ones8 = misc.tile([128, 2, 128], F8)
        nc.vector.memset(ones8, 1.0)
        eps_t = misc.tile([128, 1], F32)
        nc.vector.memset(eps_t, EPS)
        eshift_t = misc.tile([128, 1], F32)
        nc.vector.memset(eshift_t, ESHIFT)
        eps128_t = misc.tile([128, 1], F32)
        nc.vector.memset(eps128_t, HD * EPS)
        gate_sb = misc.tile([128, KT], F32)
        vb_sb = misc.tile([128, KT], F32)
        rrk_all = misc.tile([128, NT, HG], F32)   # scaled k-rms reciprocals
        if has_norm_w:
            wqn_b = misc.tile([128, HG, HD], F16)
            wkn_b = misc.tile([128, HG, HD], F16)
        cs_tiles = []
        for m in range(NT):
            ct = misc.tile([128, RD // 2], F16, tag=f"cos_{m}", name=f"cos_{m}")
            st = misc.tile([128, RD // 2], F16, tag=f"sin_{m}", name=f"sin_{m}")
            cs_tiles.append((ct, st))

        # ======== RIGHT stack: big natural-layout tensors (B..E lifetimes) =========
        v_cm = tc.tile_pool(name="vpool", bufs=1, side="right")
        v_p = v_cm.__enter__()
        vnat = v_p.tile([128, NT, GCOLS], F8)
        if has_qkv_bias:
            vnat16 = v_p.tile([128, NT, GCOLS], F16)
        natqk_cm = tc.tile_pool(name="natqk", bufs=1, side="right")
        natqk = natqk_cm.__enter__()
        qnat = natqk.tile([128, NT, GCOLS], F16)
        knat = natqk.tile([128, NT, GCOLS], F16)

        # ======== phase A: layernorm + modulation + transpose -> xnT (fp8) =========
        xnT_cm = tc.tile_pool(name="xnT", bufs=1, side="left")
        xnT_p = xnT_cm.__enter__()
        xnT = xnT_p.tile([128, KT, S], F16)  # [d_in_tile, d_tile, s]

        avec_cm = tc.tile_pool(name="phA_vec", bufs=1, side="left")
        avec = avec_cm.__enter__()
        s1pc = avec.tile([128, KT], F32)
        bmc = avec.tile([128, KT], F32)
        if has_qkv_bias:
            bq_b = avec.tile([128, GCOLS], F16)
            nc.sync.dma_start(out=bq_b, in_=_bcast_from_dram(bq, 128))
            bk_b = avec.tile([128, GCOLS], F16)
            nc.sync.dma_start(out=bk_b, in_=_bcast_from_dram(bk, 128))
            bv_b = avec.tile([128, GCOLS], F16)
            nc.sync.dma_start(out=bv_b, in_=_bcast_from_dram(bv, 128))

        xall_cm = tc.tile_pool(name="phA_x", bufs=1, side="left")
        xall = xall_cm.__enter__()
        xts = []
        for i in range(NT):
            xt = xall.tile([128, D], F16, tag=f"xt{i}", name=f"xt{i}")
            nc.sync.dma_start(out=xt, in_=x[i * 128:(i + 1) * 128, :])
            xts.append(xt)
        nc.sync.dma_start(out=s1pc, in_=scale1p)
        nc.sync.dma_start(out=bmc, in_=biasm)

        a_small_cm = tc.tile_pool(name="phA_small", bufs=4, side="left")
        a_small = a_small_cm.__enter__()
        for i in range(NT):
            xt = xts[i]
            stats = a_small.tile([128, 4, 6], F32, tag="stats")
            xv = xt.rearrange("p (c f) -> p c f", c=4)
            for c in range(4):
                nc.vector.bn_stats(out=stats[:, c, :], in_=xv[:, c, :])
            mv = a_small.tile([128, 2], F32, tag="mv")
            nc.vector.bn_aggr(out=mv, in_=stats)
            rstd = a_small.tile([128, 1], F32, tag="rstd")
            nc.scalar.activation(out=rstd, in_=mv[:, 1:2], func=AF.Sqrt,
                                 bias=eps_t, scale=1.0)
            nc.vector.reciprocal(out=rstd, in_=rstd)
            # x_norm = (x - mean) * rstd, in place on DVE (fp16 2x mode)
            nc.vector.tensor_scalar(out=xt, in0=xt, scalar1=mv[:, 0:1],
                                    scalar2=rstd, op0=OP.subtract, op1=OP.mult)

        # transposes, i-major over two 8-bank halves: each i-pass runs as soon
        # as tile i's layernorm lands, so the in-order PE tracks the DVE
        # instead of blocking on the last tile. 8 s-blocks of one d-tile pack
        # into one PSUM bank; evacs alternate ACT/DVE.
        pst_cm = tc.tile_pool(name="ps_tr", bufs=1, space="PSUM")
        pst = pst_cm.__enter__()
        for half in range(2):
            pts = [pst.tile([128, NT, 128], F16, tag=f"pt{kk}", name=f"pt{kk}")
                   for kk in range(8)]
            for i in range(NT):
                for kk in range(8):
                    k = half * 8 + kk
                    nc.tensor.matmul(pts[kk][:, i, :],
                                     xts[i][:, k * 128:(k + 1) * 128],
                                     ident, is_transpose=True,
                                     start=(i == 0), stop=(i == NT - 1),
                                     skip_group_check=True)
            # modulation fused into the evac: xnT = pt * (1+scale[d]) + bias[d]
            for kk in range(8):
                k = half * 8 + kk
                ptf = pts[kk].rearrange("p a b -> p (a b)")
                if kk % 2 == 0:
                    nc.scalar.activation(out=xnT[:, k, :], in_=ptf,
                                         func=AF.Identity,
                                         bias=bmc[:, k:k + 1],
                                         scale=s1pc[:, k:k + 1])
                else:
                    nc.vector.tensor_scalar(out=xnT[:, k, :], in0=ptf,
                                            scalar1=s1pc[:, k:k + 1],
                                            scalar2=bmc[:, k:k + 1],
                                            op0=OP.mult, op1=OP.add)
        pst_cm.__exit__(None, None, None)
        a_small_cm.__exit__(None, None, None)
        xall_cm.__exit__(None, None, None)

        # phase C pools opened BEFORE phase B emission so the rms/rope work can
        # overlap the v-projection matmuls (no pool-boundary serialization).
        c_cm = tc.tile_pool(name="phC", bufs=2, side="left")
        c_p = c_cm.__enter__()
        c_small_cm = tc.tile_pool(name="phC_small", bufs=2, side="left")
        c_small = c_small_cm.__enter__()

        # ======== phase B: qkv projections (fp16) ==================================
        wv_cm = tc.tile_pool(name="wv_res", bufs=1, side="right")
        wv_p = wv_cm.__enter__()
        w_cm = tc.tile_pool(name="wstream", bufs=1, side="right")
        w_p = w_cm.__enter__()
        psb_cm = tc.tile_pool(name="ps_qkv", bufs=1, space="PSUM")
        psb = psb_cm.__enter__()

        def emit_proj(wdram, evacs, morder=None, after_n1_evac=None, wtag="wt"):
            """m-major: each m's K-chain completes early so its consumer
            (PSUM evac + per-m epilogue) pipelines with the next chain."""
            wts = []
            for k in range(KT):
                wt = w_p.tile([128, GCOLS], F16, tag=f"{wtag}{k}", name=f"{wtag}{k}")
                nc.sync.dma_start(out=wt, in_=wdram[k])
                wts.append(wt)
            for n in range(2):
                for m in (morder or range(NT)):
                    ps = psb.tile([128, 512], F32, tag=f"ps{m}", name=f"ps{m}")
                    for k in range(KT):
                        nc.tensor.matmul(
                            ps, xnT[:, k, m * 128:(m + 1) * 128],
                            wts[k][:, n * 512:(n + 1) * 512],
                            start=(k == 0), stop=(k == KT - 1))
                    evacs(n, m, ps)
                    if n == 1 and after_n1_evac is not None:
                        after_n1_evac(m)

        # ======== phase C: rmsnorm + rope on q, k (natural, in place) ==============
        # C(m) is emitted immediately after k-tile m's last PSUM evacuation, and
        # its work is split DVE/GPSIMD, so it overlaps the B matmuls instead of
        # serializing behind them on one engine.
        def emit_c(m):
            qm = qnat[:, m, :]
            km = knat[:, m, :]
            (ct, st) = cs_tiles[m]
            cb = ct.unsqueeze(1).broadcast_to([128, HG, RD // 2])
            sb_ = st.unsqueeze(1).broadcast_to([128, HG, RD // 2])
            if has_qkv_bias:
                nc.vector.tensor_add(out=qm, in0=qm, in1=bq_b)
                nc.vector.tensor_add(out=km, in0=km, in1=bk_b)

            # rms stats (on raw q/k, before norm-w and rope)
            sq = c_p.tile([128, GCOLS], F16, tag="sqk")
            nc.vector.tensor_mul(out=sq, in0=qm, in1=qm)
            ssq = c_small.tile([128, HG], F16, tag="ssq")
            nc.vector.reduce_sum(out=ssq, in_=sq.rearrange("p (h d) -> p h d", h=HG),
                                 axis=mybir.AxisListType.X)
            rrq = c_small.tile([128, HG], F32, tag="rrq")
            nc.scalar.activation(out=rrq, in_=ssq, func=AF.Sqrt,
                                 bias=eps_t, scale=1.0 / HD)
            nc.vector.reciprocal(out=rrq, in_=rrq)
            rrq16 = c_small.tile([128, HG], F16, tag="rrq16")
            nc.vector.tensor_copy(out=rrq16, in_=rrq)

            sk_ = c_p.tile([128, GCOLS], F16, tag="sqk")
            nc.vector.tensor_mul(out=sk_, in0=km, in1=km)
            ssk = c_small.tile([128, HG], F16, tag="ssk")
            nc.vector.reduce_sum(out=ssk, in_=sk_.rearrange("p (h d) -> p h d", h=HG),
                                 axis=mybir.AxisListType.X)
            nc.scalar.activation(out=rrk_all[:, m, :], in_=ssk, func=AF.Sqrt,
                                 bias=eps128_t, scale=1.0)
            nc.vector.reciprocal(out=rrk_all[:, m, :], in_=rrk_all[:, m, :])

            if has_norm_w:
                nc.vector.tensor_mul(out=qm.rearrange("p (h d) -> p h d", h=HG),
                                     in0=qm.rearrange("p (h d) -> p h d", h=HG),
                                     in1=wqn_b)
                nc.vector.tensor_mul(out=km.rearrange("p (h d) -> p h d", h=HG),
                                     in0=km.rearrange("p (h d) -> p h d", h=HG),
                                     in1=wkn_b)

            # rope with host-permuted pair layout: x0 = cols [0:32), x1 = [32:64)
            # of each head (evens/odds separated on host; scores invariant).
            # q rope on DVE, k rope on GPSIMD (runs in parallel).
            for mm, eng in ((qm, nc.vector), (km, nc.gpsimd)):
                mv_ = mm.rearrange("p (h d) -> p h d", h=HG)
                x0 = mv_[:, :, 0:RD // 2]
                x1 = mv_[:, :, RD // 2:RD]
                tg = "q" if eng is nc.vector else "k"
                t0 = c_small.tile([128, HG, RD // 2], F16, tag=f"t0{tg}")
                t1 = c_small.tile([128, HG, RD // 2], F16, tag=f"t1{tg}")
                t2 = c_small.tile([128, HG, RD // 2], F16, tag=f"t2{tg}")
                t3 = c_small.tile([128, HG, RD // 2], F16, tag=f"t3{tg}")
                eng.tensor_mul(out=t0, in0=x0, in1=cb)
                eng.tensor_mul(out=t1, in0=x1, in1=sb_)
                eng.tensor_mul(out=t2, in0=x0, in1=sb_)
                eng.tensor_mul(out=t3, in0=x1, in1=cb)
                eng.tensor_sub(out=x0, in0=t0, in1=t1)
                eng.tensor_add(out=x1, in0=t2, in1=t3)

            # apply q rms reciprocal (k's is folded into the exp scale later)
            rrq_b = rrq16.unsqueeze(2).broadcast_to([128, HG, HD])
            nc.gpsimd.tensor_mul(out=qm.rearrange("p (h d) -> p h d", h=HG),
                                 in0=qm.rearrange("p (h d) -> p h d", h=HG),
                                 in1=rrq_b)

        # rotate the first group's m-order: ps[0]/ps[1] reuse the PSUM banks of
        # the last two transpose tiles, whose evacs land latest
        emit_proj(wq, lambda n, m, ps: nc.scalar.copy(
            out=qnat[:, m, n * 512:(n + 1) * 512], in_=ps), wtag="wq")

        # deferred misc loads (consumed in phases C/E/F) -- emitted after the
        # q weight stream so their HWDGE overhead doesn't delay phase B's start
        nc.sync.dma_start(out=gate_sb, in_=gate)
        nc.sync.dma_start(out=vb_sb, in_=vb)
        if has_norm_w:
            nc.sync.dma_start(out=wqn_b, in_=_bcast_from_dram(wqn, 128, reps=HG))
            nc.sync.dma_start(out=wkn_b, in_=_bcast_from_dram(wkn, 128, reps=HG))
        for m in range(NT):
            ct, st = cs_tiles[m]
            nc.sync.dma_start(out=ct, in_=cos[m * 128:(m + 1) * 128, :])
            nc.sync.dma_start(out=st, in_=sin[m * 128:(m + 1) * 128, :])

        emit_proj(wk, lambda n, m, ps: nc.scalar.copy(
            out=knat[:, m, n * 512:(n + 1) * 512], in_=ps),
            after_n1_evac=emit_c, wtag="wk")

        if has_qkv_bias:
            emit_proj(wv, lambda n, m, ps: nc.scalar.copy(
                out=vnat16[:, m, n * 512:(n + 1) * 512], in_=ps), wtag="wv")
            for m in range(NT):
                nc.vector.tensor_add(out=vnat16[:, m, :], in0=vnat16[:, m, :],
                                     in1=bv_b)
                nc.vector.tensor_copy(out=vnat[:, m, :], in_=vnat16[:, m, :])
            wvts = None
        else:
            # v weights stay resident; the v projection is emitted later as
            # 2-PSUM-bank m-serial chains (n=0 before phase D, n=1 woven into
            # phase E's head loop to fill PE stalls during the exp waits).
            wvts = []
            for k in range(KT):
                wt = wv_p.tile([128, GCOLS], F16, tag=f"wv{k}", name=f"wv{k}")
                nc.sync.dma_start(out=wt, in_=wv[k])
                wvts.append(wt)

        psb_cm.__exit__(None, None, None)
        w_cm.__exit__(None, None, None)

        psbv_cm = tc.tile_pool(name="ps_v", bufs=2, space="PSUM")
        psbv = psbv_cm.__enter__()

        def bv_thunks(n):
            """Yield emission thunks for the v projection of one n-half:
            per m, a 16-matmul K-chain into a single PSUM bank + DVE evac."""
            for m in range(NT):
                psv = psbv.tile([128, 512], F32, tag="psv")
                for k in range(KT):
                    yield (lambda psv=psv, k=k, m=m, n=n: nc.tensor.matmul(
                        psv, xnT[:, k, m * 128:(m + 1) * 128],
                        wvts[k][:, n * 512:(n + 1) * 512],
                        start=(k == 0), stop=(k == KT - 1)))
                # n=0 evacs on ACT (idle pre-E); n=1 evacs on DVE (idle in D)
                if n == 0:
                    yield (lambda psv=psv, m=m, n=n: nc.scalar.copy(
                        out=vnat[:, m, n * 512:(n + 1) * 512], in_=psv))
                else:
                    yield (lambda psv=psv, m=m, n=n: nc.vector.tensor_copy(
                        out=vnat[:, m, n * 512:(n + 1) * 512], in_=psv))

        if not has_qkv_bias:
            for t in bv_thunks(0):
                t()
            bv_rest = bv_thunks(1)
        else:
            bv_rest = iter(())

        def drain(nthunks):
            for _ in range(nthunks):
                t = next(bv_rest, None)
                if t is None:
                    return
                t()

        # ======== phase D: transpose q, k -> [hd, s] per head ======================
        oT_cm = tc.tile_pool(name="oT", bufs=1, side="left")
        oT_p = oT_cm.__enter__()
        oT = oT_p.tile([128, HG, S], F16)

        qkT_cm = tc.tile_pool(name="qkT", bufs=1, side="left")
        qkT_p = qkT_cm.__enter__()
        qT = qkT_p.tile([128, HG, S], F16)
        kT = qkT_p.tile([128, HG, S], F16)

        # the v projection's n=1 half is woven between the transpose groups:
        # the PE is otherwise evac-paced here, and psbv (2 banks) + pst2 (4)
        # fit alongside each other before the attention pools open
        pst2_cm = tc.tile_pool(name="ps_tr2", bufs=4, space="PSUM")
        pst2 = pst2_cm.__enter__()
        for h in range(HG):
            for (nat, dst, evac) in (
                    (qnat, qT, lambda o, i: nc.scalar.copy(out=o, in_=i)),
                    (knat, kT, lambda o, i: nc.vector.tensor_copy(out=o, in_=i))):
                pt2 = pst2.tile([128, NT, 128], F16, tag="pt2")
                for m in range(NT):
                    nc.tensor.matmul(pt2[:, m, :], nat[:, m, h * 128:(h + 1) * 128],
                                     ident, is_transpose=True,
                                     start=(m == 0), stop=(m == NT - 1),
                                     skip_group_check=True)
                drain(9)
                evac(dst[:, h, :], pt2.rearrange("p a b -> p (a b)"))
        drain(1 << 30)
        pst2_cm.__exit__(None, None, None)
        psbv_cm.__exit__(None, None, None)

        # ======== phase E: attention per head ======================================
        at_cm = tc.tile_pool(name="attn", bufs=2, side="left")
        at_p = at_cm.__enter__()
        rs_cm = tc.tile_pool(name="rsb", bufs=2, side="left")
        rs_p = rs_cm.__enter__()
        pssc_cm = tc.tile_pool(name="ps_sc", bufs=2, space="PSUM")
        pssc = pssc_cm.__enter__()
        pso_cm = tc.tile_pool(name="ps_o", bufs=1, space="PSUM")
        pso = pso_cm.__enter__()
        psd_cm = tc.tile_pool(name="ps_den", bufs=1, space="PSUM")
        psd = psd_cm.__enter__()

        for h in range(HG):
            o_ps = pso.tile([128, S], F32, tag="o_ps")
            # ones-matmul denominator: every partition gets the same column sums,
            # so no partition broadcast is needed afterwards
            den_ps = psd.tile([128, 2, 512], F32, tag="den_ps")
            at2 = None
            sc_next = pssc.tile([128, S], F32, tag="sc")
            nc.tensor.matmul(sc_next[:, 0:512], kT[:, h, 0:128], qT[:, h, 0:512],
                             start=True, stop=True)
            nc.tensor.matmul(sc_next[:, 512:1024], kT[:, h, 0:128],
                             qT[:, h, 512:1024], start=True, stop=True)
            for m in range(NT):
                sc = sc_next
                if m < NT - 1:
                    sc_next = pssc.tile([128, S], F32, tag="sc")
                    lhs_k = kT[:, h, (m + 1) * 128:(m + 2) * 128]
                    nc.tensor.matmul(sc_next[:, 0:512], lhs_k, qT[:, h, 0:512],
                                     start=True, stop=True)
                    nc.tensor.matmul(sc_next[:, 512:1024], lhs_k,
                                     qT[:, h, 512:1024], start=True, stop=True)
                if m % 2 == 0:
                    at2 = at_p.tile([128, 2, S], F8, tag="at2", name="at2")
                # attn weights: exp(score/rms - 2), fp8 (shift cancels in softmax)
                nc.scalar.activation(out=at2[:, m % 2, :], in_=sc, func=AF.Exp,
                                     bias=eshift_t, scale=rrk_all[:, m, h:h + 1])
                if m % 2 == 1:
                    mp = m // 2
                    first, last = (mp == 0), (mp == MP - 1)
                    v_mh = vnat[:, m - 1:m + 1, h * 128:(h + 1) * 128]
                    for c in range(2):
                        rhs = at2[:, :, c * 512:(c + 1) * 512]
                        nc.tensor.matmul(o_ps[:, c * 512:(c + 1) * 512], v_mh,
                                         rhs, start=first, stop=last, perf_mode=DR)
                        nc.tensor.matmul(den_ps[:, c, :], ones8, rhs,
                                         start=first, stop=last, perf_mode=DR)
            # reciprocal of the (partition-replicated) denominator, normalize
            sums_b = rs_p.tile([128, S], F16, tag="sums_b")
            nc.vector.reciprocal(out=sums_b,
                                 in_=den_ps.rearrange("p a b -> p (a b)"))
            nc.vector.tensor_mul(out=oT[:, h, :], in0=o_ps, in1=sums_b)

        psd_cm.__exit__(None, None, None)
        pso_cm.__exit__(None, None, None)
        pssc_cm.__exit__(None, None, None)
        rs_cm.__exit__(None, None, None)
        at_cm.__exit__(None, None, None)
        qkT_cm.__exit__(None, None, None)
        wv_cm.__exit__(None, None, None)
        natqk_cm.__exit__(None, None, None)
        v_cm.__exit__(None, None, None)

        # ---- phase F: out projection (transposed out)
        f_cm = tc.tile_pool(name="phF", bufs=2, side="left")
        f_p = f_cm.__enter__()
        psf_cm = tc.tile_pool(name="ps_out", bufs=2, space="PSUM")
        psf = psf_cm.__enter__()
        for mp in range(KT // 2):
            wo_t = f_p.tile([128, 2, GCOLS], F16, tag="wo_t")
            nc.sync.dma_start(out=wo_t,
                              in_=wo.rearrange("m p c -> p m c")[:, 2 * mp:2 * mp + 2, :])
            ot2 = f_p.tile([128, 2, S], F16, tag="ot2")
            for j in range(2):
                m = 2 * mp + j
                po = psf.tile([128, S], F32, tag="po")
                for kb in range(HG):
                    first, last = (kb == 0), (kb == HG - 1)
                    lhs = wo_t[:, j, kb * 128:(kb + 1) * 128]
                    nc.tensor.matmul(po[:, 0:512], lhs, oT[:, kb, 0:512],
                                     start=first, stop=last)
                    nc.tensor.matmul(po[:, 512:1024], lhs, oT[:, kb, 512:1024],
                                     start=first, stop=last)
                nc.scalar.activation(out=ot2[:, j, :], in_=po, func=AF.Identity,
                                     bias=vb_sb[:, m:m + 1], scale=gate_sb[:, m:m + 1])
            nc.sync.dma_start(out=out_r[:, 2 * mp:2 * mp + 2, :], in_=ot2)
        psf_cm.__exit__(None, None, None)
        f_cm.__exit__(None, None, None)
        oT_cm.__exit__(None, None, None)
        c_small_cm.__exit__(None, None, None)
        c_cm.__exit__(None, None, None)
        avec_cm.__exit__(None, None, None)
        xnT_cm.__exit__(None, None, None)
        misc_cm.__exit__(None, None, None)

    nc.compile()
    return nc


_NC_CACHE = {}


def _get_nc(has_qkv_bias, has_norm_w):
    key = (has_qkv_bias, has_norm_w)
    if key not in _NC_CACHE:
        _NC_CACHE[key] = build_nc(*key)
    return _NC_CACHE[key]


# rope pair permutation: within each head's first RD columns, evens then odds.
_ROPE_PERM_HD = np.r_[np.arange(0, RD, 2), np.arange(1, RD, 2), np.arange(RD, HD)]
_ROPE_PERM = np.concatenate([h * HD + _ROPE_PERM_HD for h in range(HG)])


def _pack_qkv_w(w):
    """[D, GCOLS] fp32 -> [KT, 128, GCOLS] fp16."""
    return np.ascontiguousarray(w.reshape(KT, 128, GCOLS).astype(np.float16))


def _pack_wo(w):
    """[GCOLS, D] fp32 -> [KT, 128, HG*128] fp16 (wo_p[m,p,kb*128+j] = w[kb*128+p, m*128+j])."""
    wp = w.reshape(HG, 128, KT, 128).transpose(2, 1, 0, 3).reshape(KT, 128, GCOLS)
    return np.ascontiguousarray(wp.astype(np.float16))


def prep_in_maps(x, mod, cos, sin, qkv_w, qkv_b, mod_w, mod_b, out_w, out_b,
                 norm_q_w, norm_k_w):
    """Host-side sharding. Returns (in_maps, flags, x_np)."""
    x = np.asarray(x, dtype=np.float32)
    m3 = np.asarray(mod, np.float32) @ np.asarray(mod_w, np.float32) \
        + np.asarray(mod_b, np.float32)
    bias, scale, gatef = np.split(m3, 3, axis=-1)          # [B, D] each
    scale1p = (1.0 + scale).astype(np.float32)
    vbf = (np.asarray(out_b, np.float32)[None, :] * gatef).astype(np.float32)

    qkv_b = np.asarray(qkv_b, np.float32)
    has_qkv_bias = bool(np.any(qkv_b != 0.0))
    has_norm_w = not (np.allclose(norm_q_w, 1.0) and np.allclose(norm_k_w, 1.0))

    cosc = np.ascontiguousarray(np.asarray(cos, np.float16))
    sinc = np.ascontiguousarray(np.asarray(sin, np.float16))
    qkv_w = np.asarray(qkv_w, np.float32)
    out_w = np.asarray(out_w, np.float32)
    x16 = x.astype(np.float16)

    in_maps = []
    for c in range(N_CORES):
        b, g = divmod(c, 2)
        lo = g * GCOLS
        im = {
            "x": np.ascontiguousarray(x16[b]),
            "cos": cosc, "sin": sinc,
            "wq": _pack_qkv_w(qkv_w[:, lo:lo + GCOLS][:, _ROPE_PERM]),
            "wk": _pack_qkv_w(qkv_w[:, 2048 + lo:2048 + lo + GCOLS][:, _ROPE_PERM]),
            "wv": _pack_qkv_w(qkv_w[:, 4096 + lo:4096 + lo + GCOLS]),
            "wo": _pack_wo(out_w[lo:lo + GCOLS, :]),
            "scale1p": np.ascontiguousarray(scale1p[b].reshape(KT, 128).T),
            "biasm": np.ascontiguousarray(bias[b].reshape(KT, 128).T),
            "gate": np.ascontiguousarray(gatef[b].reshape(KT, 128).T),
            "vb": np.ascontiguousarray(
                (vbf[b] if g == 0 else np.zeros_like(vbf[b])).reshape(KT, 128).T),
        }
        if has_qkv_bias:
            im["bq"] = np.ascontiguousarray(
                qkv_b[lo:lo + GCOLS][_ROPE_PERM].astype(np.float16))
            im["bk"] = np.ascontiguousarray(
                qkv_b[2048 + lo:2048 + lo + GCOLS][_ROPE_PERM].astype(np.float16))
            im["bv"] = np.ascontiguousarray(
                qkv_b[4096 + lo:4096 + lo + GCOLS].astype(np.float16))
        if has_norm_w:
            im["wqn"] = np.ascontiguousarray(
                np.asarray(norm_q_w, np.float32)[_ROPE_PERM_HD].astype(np.float16))
            im["wkn"] = np.ascontiguousarray(
                np.asarray(norm_k_w, np.float32)[_ROPE_PERM_HD].astype(np.float16))
        in_maps.append(im)
    return in_maps, (has_qkv_bias, has_norm_w), x


def gather(results, x):
    B = x.shape[0]
    outs = []
    for b in range(B):
        p = results[2 * b]["out_t"].astype(np.float32) \
            + results[2 * b + 1]["out_t"].astype(np.float32)   # [D, S]
        outs.append(p.T + x[b])
    return np.stack(outs).astype(np.float32)


def kernel(**inputs) -> np.ndarray:
    in_maps, flags, x = prep_in_maps(**inputs)
    nc = _get_nc(*flags)
    res = run_bass_kernel_spmd(nc, in_maps, core_ids=list(range(N_CORES)))
    return gather(res.results, x)


if __name__ == "__main__":
    import time
    t0 = time.time()
    nc = build_nc(False, False)
    print("build+compile ok in", time.time() - t0, "s")
